# revision 23
# baseline (speedup 1.0000x reference)
"""Trainium2 Bass kernel for BinarizeConv2dSDP.

Reference math (forward only):
    w    = rsqrt(m^2 + sum_k z_k^2/100) * (m + rv @ z)   elementwise
    bw   = sign(w)        -- the positive rsqrt factor drops out of sign()
    ba   = sign(x)
    out  = conv2d(ba, bw, pad=1, NCHW/OIHW) * alpha[o]

Device computation: bw = sign(M + sum_k rv[k]*Z[k]), ba = sign(x), then the
3x3 pad-1 conv as 9 shifted fp8 DoubleRow matmuls accumulating in PSUM
(everything is +-1, so fp8 e4m3 with f32 PSUM accumulation is bit-exact),
alpha folded into the PSUM->SBUF copy.

Sharding (8 cores, no collectives): 2D grid, batch 4-way x out-channel
2-way. Core i handles images [16*(i//2), 16*(i//2)+16) and out-channels
[128*(i%2), 128*(i%2)+128). Each core reads only its Z/M/alpha o-half and
its x batch-quarter; outputs are disjoint.

Per-core layouts:
  - z_k, m, wsum: [128 part(o), 2304 (c*9+t)] f32 -- natural Z order, so
    all weight DMAs are fully contiguous.
  - weight sum on ACT (muls by rv[k]) + DVE (sequential add chain, same
    f32 order as the reference dot).
  - sign -> w8 [128(o), 2304] fp8; 18 PE transposes (matmul with fp8
    identity rhs, lhsT = stride-9 column slice) -> W [128 part(c_low),
    9 tap, 2 c-chunk, 128 o] fp8.
  - activations: per image [128 part(c_low), 2 c-chunk, 912] fp8 zero-
    padded 30x30 images (912 = 900 rounded up to %16 for the DoubleRow
    AP-step constraint); conv output on a 30-wide grid, junk columns
    skipped at the output DMA.
"""

import sys

for _p in ("/opt/trn_rl_repo",):
    if _p not in sys.path:
        sys.path.insert(0, _p)

import contextlib

import numpy as np

import concourse.bass as bass
import concourse.bacc as bacc
import concourse.tile as tile
from concourse import mybir
from concourse.bass_utils import run_bass_kernel_spmd

N_CORES = 8
B = 64
B_SH = 16       # images per core (batch/4)
C = 256         # in channels
O = 256
O_SH = 128      # out channels per core (o/2)
K = 8           # SDP rank
KK = 9          # 3x3 taps
CT = C * KK     # 2304
H = 28
HP = 30         # padded row width
PADW = 912      # 30*30=900 padded to %16
F32 = mybir.dt.float32
FP8 = mybir.dt.float8e4


def _build_kernel(tc, x_t, m_t, z_t, a_t, rv_t, eye_t, ones_t, out_t):
    nc = tc.nc
    ctx = contextlib.ExitStack()
    consts = ctx.enter_context(tc.tile_pool(name="consts", bufs=1))
    zpool = ctx.enter_context(tc.tile_pool(name="zpool", bufs=1))
    wpool = ctx.enter_context(tc.tile_pool(name="wpool", bufs=1))
    stage = ctx.enter_context(tc.tile_pool(name="stage", bufs=8))
    acts = ctx.enter_context(tc.tile_pool(name="acts", bufs=1))
    outp = ctx.enter_context(tc.tile_pool(name="outp", bufs=8))
    psums = ctx.enter_context(tc.tile_pool(name="psums", bufs=6, space="PSUM"))
    pst = ctx.enter_context(tc.tile_pool(name="pst", bufs=2, space="PSUM"))

    with ctx:
        # ---- tiny constants. rv is partition-broadcast via a K=1 matmul
        # (ones.T @ rv) on the otherwise-idle PE — a [0,128]-step broadcast
        # DMA would stall its queue with 128 tiny descriptors. ----
        rv_raw = consts.tile([1, K], F32, name="rv_raw")
        nc.sync.dma_start(rv_raw, rv_t.ap())
        alpha_sb = consts.tile([128, 1], F32, name="alpha_sb")
        nc.sync.dma_start(alpha_sb, a_t.ap().rearrange("p a b -> p (a b)"))
        ones_sb = consts.tile([1, 128], F32, name="ones_sb")
        nc.sync.dma_start(ones_sb, ones_t.ap())
        ps_rv = pst.tile([128, K], F32, name="ps_rv", tag="ps_t")
        nc.tensor.matmul(ps_rv, ones_sb, rv_raw, start=True, stop=True)
        rv_sb = consts.tile([128, K], F32, name="rv_sb")
        nc.vector.tensor_copy(rv_sb, ps_rv)
        eye_sb = consts.tile([128, 128], F32, name="eye_sb")
        nc.gpsimd.dma_start(eye_sb, eye_t.ap())
        eye8 = consts.tile([128, 128], FP8, name="eye8")
        nc.scalar.sign(eye8, eye_sb)

        # ---- weight inputs first: fully contiguous [o, c*9+t] loads.
        # Everything big goes on the single SP HWDGE ring, in priority
        # order (z+M gate the conv start; x streams behind them). ----
        HCT = CT // 2
        m_sb = zpool.tile([128, CT], F32, name="m_sb")
        nc.sync.dma_start(m_sb, m_t.ap().rearrange("o c ky kx -> o (c ky kx)"))
        z_sb = []
        for k in range(K):
            z_k = zpool.tile([128, CT], F32, name=f"z{k}", tag="z", bufs=5)
            z_src = z_t.ap()[k].rearrange("o c ky kx -> o (c ky kx)")
            for h in range(2):
                sl = slice(h * HCT, (h + 1) * HCT)
                nc.sync.dma_start(z_k[:, sl], z_src[:, sl])
            z_sb.append(z_k)

        xst = []
        for n in range(B_SH):
            xst.append(stage.tile([128, 2, H * H], F32, name=f"xst{n}", tag="xst"))
        for n in range(B_SH):
            nc.sync.dma_start(
                xst[n], x_t.ap()[n].rearrange("(cc p) h w -> p cc (h w)", p=128)
            )

        # ---- zero activation-padding borders early (DVE, cheap) so the
        # ACT signs are gated only by the x arrivals ----
        act_tiles = []
        for n in range(B_SH):
            a_n = acts.tile([128, 2, PADW], FP8, name=f"a{n}", tag=f"a{n}")
            nc.vector.memset(a_n[:, :, 0:30], 0.0)
            nc.vector.memset(a_n[:, :, 870:PADW], 0.0)
            pairs = a_n[:, :, 29 : 29 + 29 * HP].rearrange(
                "p cc (r two) -> p cc r two", two=HP
            )[:, :, :, :2]
            nc.vector.memset(pairs, 0.0)
            act_tiles.append(a_n)

        # ---- wsum = M + sum_k rv[k]*Z[k]; muls on ACT, add chain on DVE
        # (sequential k order to match the reference dot's rounding) ----
        # FMA chain, binarize, and transpose, pipelined by column halves
        # (half h of z_k arrives before the halves of z_{k+1})
        acc = wpool.tile([128, CT], F32, name="acc")
        w8 = wpool.tile([128, CT], FP8, name="w8")
        wt = consts.tile([128, KK, 2, 128], FP8, name="wt")
        halves = (slice(0, HCT), slice(HCT, CT))
        for h in range(2):
            sl = halves[h]
            nc.vector.tensor_scalar_mul(acc[:, sl], z_sb[0][:, sl], rv_sb[:, 0:1])
            for k in range(1, K):
                nc.vector.scalar_tensor_tensor(
                    acc[:, sl],
                    z_sb[k][:, sl],
                    rv_sb[:, k : k + 1],
                    acc[:, sl],
                    op0=mybir.AluOpType.mult,
                    op1=mybir.AluOpType.add,
                )
            nc.vector.tensor_add(acc[:, sl], acc[:, sl], m_sb[:, sl])
            nc.scalar.sign(w8[:, sl], acc[:, sl])
            cc = h  # c-chunk cc reads w8 columns [cc*1152, cc*1152+1152)
            for t in range(KK):
                blk = bass.AP(
                    tensor=w8.tensor,
                    offset=w8.offset + cc * 128 * KK + t,
                    ap=[w8.ap[0], [KK, 128]],
                )
                ps_t = pst.tile([128, 128], F32, name="ps_t", tag="ps_t")
                nc.tensor.matmul(ps_t, blk, eye8, start=True, stop=True)
                nc.vector.tensor_copy(wt[:, t, cc, :], ps_t)

        # ---- activations: sign(x) into the padded fp8 images (ACT) ----
        for n in range(B_SH):
            a_n = act_tiles[n]
            interior = a_n[:, :, 31 : 31 + 28 * HP].rearrange(
                "p cc (r xx) -> p cc r xx", xx=HP
            )[:, :, :, :28]
            nc.scalar.sign(interior, xst[n].rearrange("p cc (h w) -> p cc h w", w=28))

        # ---- conv: 9 taps x 2 half-images per image; both halves share
        # each tap's LDWEIGHTS (pair the matmuls) so weight loads hide ----
        for n in range(B_SH):
            a_n = act_tiles[n]
            ps0 = psums.tile([128, 420], F32, name="ps0", tag="ps")
            ps1 = psums.tile([128, 420], F32, name="ps1", tag="ps")
            pss = (ps0, ps1)
            for t in range(KK):
                dy, dx = divmod(t, 3)
                for half in range(2):
                    off = (half * 14 + dy) * HP + dx
                    nc.tensor.matmul(
                        pss[half],
                        wt[:, t],
                        a_n[:, :, off : off + 420],
                        start=(t == 0),
                        stop=(t == KK - 1),
                        perf_mode=mybir.MatmulPerfMode.DoubleRow,
                    )
            for half in range(2):
                ob = outp.tile([128, 392], F32, name="ob", tag="ob")
                ps_v = pss[half].rearrange("p (r xx) -> p r xx", xx=HP)[:, :, :28]
                ob_v = ob.rearrange("p (r xx) -> p r xx", xx=28)
                # alternate the psum-drain engine to balance ACT/DVE
                if half == 0:
                    nc.scalar.activation(
                        ob_v,
                        ps_v,
                        mybir.ActivationFunctionType.Copy,
                        scale=alpha_sb[:, 0:1],
                    )
                else:
                    nc.vector.tensor_scalar_mul(ob_v, ps_v, alpha_sb[:, 0:1])
                dst = out_t.ap()[n].rearrange("o h w -> o (h w)")[
                    :, half * 392 : (half + 1) * 392
                ]
                # out-writes ride the ACT HWDGE ring; the SP ring stays
                # dedicated to the input stream
                nc.scalar.dma_start(dst, ob)


_PROGRAM = None


def build_program():
    global _PROGRAM
    if _PROGRAM is not None:
        return _PROGRAM
    nc = bacc.Bacc(
        "TRN2",
        target_bir_lowering=False,
        debug=False,
        enable_asserts=True,
        num_devices=N_CORES,
    )
    x_t = nc.dram_tensor("x", [B_SH, C, H, H], F32, kind="ExternalInput")
    m_t = nc.dram_tensor("M", [O_SH, C, 3, 3], F32, kind="ExternalInput")
    z_t = nc.dram_tensor("Z", [K, O_SH, C, 3, 3], F32, kind="ExternalInput")
    a_t = nc.dram_tensor("alpha", [O_SH, 1, 1], F32, kind="ExternalInput")
    rv_t = nc.dram_tensor("rv", [1, K], F32, kind="ExternalInput")
    eye_t = nc.inline_tensor(np.eye(128, dtype=np.float32), name="eye128")
    ones_t = nc.inline_tensor(np.ones((1, 128), dtype=np.float32), name="ones128")
    out_t = nc.dram_tensor("out", [B_SH, O_SH, H, H], F32, kind="ExternalOutput")

    with tile.TileContext(nc) as tc:
        _build_kernel(tc, x_t, m_t, z_t, a_t, rv_t, eye_t, ones_t, out_t)
    nc.compile()
    _PROGRAM = nc
    return nc


def make_in_maps(x, M, Z, alpha, rv):
    x = np.ascontiguousarray(np.asarray(x, dtype=np.float32))
    M = np.ascontiguousarray(np.asarray(M, dtype=np.float32))
    Z = np.ascontiguousarray(np.asarray(Z, dtype=np.float32))
    alpha = np.ascontiguousarray(np.asarray(alpha, dtype=np.float32))
    rv = np.ascontiguousarray(np.asarray(rv, dtype=np.float32))
    in_maps = []
    for i in range(N_CORES):
        b, oh = divmod(i, 2)
        in_maps.append(
            {
                "x": np.ascontiguousarray(x[b * B_SH : (b + 1) * B_SH]),
                "M": np.ascontiguousarray(M[oh * O_SH : (oh + 1) * O_SH]),
                "Z": np.ascontiguousarray(Z[:, oh * O_SH : (oh + 1) * O_SH]),
                "alpha": np.ascontiguousarray(alpha[oh * O_SH : (oh + 1) * O_SH]),
                "rv": rv,
            }
        )
    return in_maps


def assemble_out(results):
    out = np.empty((B, O, H, H), dtype=np.float32)
    for i in range(N_CORES):
        b, oh = divmod(i, 2)
        r = np.asarray(results[i]["out"]).reshape(B_SH, O_SH, H, H)
        out[b * B_SH : (b + 1) * B_SH, oh * O_SH : (oh + 1) * O_SH] = r
    return out


def kernel(x, M, Z, alpha, rv, trace=False):
    nc = build_program()
    in_maps = make_in_maps(x, M, Z, alpha, rv)
    res = run_bass_kernel_spmd(
        nc, in_maps, core_ids=list(range(N_CORES)), trace=trace
    )
    if trace:
        kernel.last_results = res
    return assemble_out(res.results)


if __name__ == "__main__":
    build_program()
    print("program built ok")


# revision 24
# speedup vs baseline: 1.1064x; 1.1064x over previous
"""Trainium2 Bass kernel for BinarizeConv2dSDP.

Reference math (forward only):
    w    = rsqrt(m^2 + sum_k z_k^2/100) * (m + rv @ z)   elementwise
    bw   = sign(w)        -- the positive rsqrt factor drops out of sign()
    ba   = sign(x)
    out  = conv2d(ba, bw, pad=1, NCHW/OIHW) * alpha[o]

Device computation: bw = sign(M + sum_k rv[k]*Z[k]), ba = sign(x), then the
3x3 pad-1 conv as 9 shifted fp8 DoubleRow matmuls accumulating in PSUM
(everything is +-1, so fp8 e4m3 with f32 PSUM accumulation is bit-exact),
alpha folded into the PSUM->SBUF copy.

Sharding (8 cores, no collectives): 2D grid, batch 4-way x out-channel
2-way. Core i handles images [16*(i//2), 16*(i//2)+16) and out-channels
[128*(i%2), 128*(i%2)+128). Each core reads only its Z/M/alpha o-half and
its x batch-quarter; outputs are disjoint.

Per-core layouts:
  - z_k, m, wsum: [128 part(o), 2304 (c*9+t)] f32 -- natural Z order, so
    all weight DMAs are fully contiguous.
  - weight sum on ACT (muls by rv[k]) + DVE (sequential add chain, same
    f32 order as the reference dot).
  - sign -> w8 [128(o), 2304] fp8; 18 PE transposes (matmul with fp8
    identity rhs, lhsT = stride-9 column slice) -> W [128 part(c_low),
    9 tap, 2 c-chunk, 128 o] fp8.
  - activations: per image [128 part(c_low), 2 c-chunk, 912] fp8 zero-
    padded 30x30 images (912 = 900 rounded up to %16 for the DoubleRow
    AP-step constraint); conv output on a 30-wide grid, junk columns
    skipped at the output DMA.
"""

import sys

for _p in ("/opt/trn_rl_repo",):
    if _p not in sys.path:
        sys.path.insert(0, _p)

import contextlib

import numpy as np

import concourse.bass as bass
import concourse.bacc as bacc
import concourse.tile as tile
from concourse import mybir
from concourse.bass_utils import run_bass_kernel_spmd

N_CORES = 8
B = 64
B_SH = 16       # images per core (batch/4)
C = 256         # in channels
O = 256
O_SH = 128      # out channels per core (o/2)
K = 8           # SDP rank
KK = 9          # 3x3 taps
CT = C * KK     # 2304
H = 28
HP = 30         # padded row width
PADW = 912      # 30*30=900 padded to %16
F32 = mybir.dt.float32
FP8 = mybir.dt.float8e4


def _build_kernel(tc, x_t, m_t, z_t, a_t, rv_t, eye_t, ones_t, out_t):
    nc = tc.nc
    ctx = contextlib.ExitStack()
    consts = ctx.enter_context(tc.tile_pool(name="consts", bufs=1))
    zpool = ctx.enter_context(tc.tile_pool(name="zpool", bufs=1))
    wpool = ctx.enter_context(tc.tile_pool(name="wpool", bufs=1))
    stage = ctx.enter_context(tc.tile_pool(name="stage", bufs=8))
    acts = ctx.enter_context(tc.tile_pool(name="acts", bufs=1))
    outp = ctx.enter_context(tc.tile_pool(name="outp", bufs=8))
    psums = ctx.enter_context(tc.tile_pool(name="psums", bufs=6, space="PSUM"))
    pst = ctx.enter_context(tc.tile_pool(name="pst", bufs=2, space="PSUM"))

    with ctx:
        # ---- tiny constants. rv is partition-broadcast via a K=1 matmul
        # (ones.T @ rv) on the otherwise-idle PE — a [0,128]-step broadcast
        # DMA would stall its queue with 128 tiny descriptors. ----
        rv_raw = consts.tile([1, K], F32, name="rv_raw")
        nc.sync.dma_start(rv_raw, rv_t.ap())
        ones_sb = consts.tile([1, 128], F32, name="ones_sb")
        nc.sync.dma_start(ones_sb, ones_t.ap())
        alpha_sb = consts.tile([128, 1], F32, name="alpha_sb")
        nc.gpsimd.dma_start(alpha_sb, a_t.ap().rearrange("p a b -> p (a b)"))
        ps_rv = pst.tile([128, K], F32, name="ps_rv", tag="ps_t")
        nc.tensor.matmul(ps_rv, ones_sb, rv_raw, start=True, stop=True)
        rv_sb = consts.tile([128, K], F32, name="rv_sb")
        nc.vector.tensor_copy(rv_sb, ps_rv)
        eye_sb = consts.tile([128, 128], F32, name="eye_sb")
        nc.gpsimd.dma_start(eye_sb, eye_t.ap())
        eye8 = consts.tile([128, 128], FP8, name="eye8")
        nc.scalar.sign(eye8, eye_sb)

        # ---- weight inputs first: fully contiguous [o, c*9+t] loads.
        # Everything big goes on the single SP HWDGE ring, in priority
        # order (z+M gate the conv start; x streams behind them). ----
        HCT = CT // 2
        m_sb = zpool.tile([128, CT], F32, name="m_sb")
        nc.sync.dma_start(m_sb, m_t.ap().rearrange("o c ky kx -> o (c ky kx)"))
        z_sb = []
        for k in range(K):
            z_k = zpool.tile([128, CT], F32, name=f"z{k}", tag="z", bufs=5)
            z_src = z_t.ap()[k].rearrange("o c ky kx -> o (c ky kx)")
            for h in range(2):
                sl = slice(h * HCT, (h + 1) * HCT)
                nc.sync.dma_start(z_k[:, sl], z_src[:, sl])
            z_sb.append(z_k)

        xst = []
        for n in range(B_SH):
            xst.append(stage.tile([128, 2, H * H], F32, name=f"xst{n}", tag="xst"))
        for n in range(B_SH):
            nc.sync.dma_start(
                xst[n], x_t.ap()[n].rearrange("(cc p) h w -> p cc (h w)", p=128)
            )

        # ---- zero activation-padding borders early (DVE, cheap) so the
        # ACT signs are gated only by the x arrivals ----
        act_tiles = []
        for n in range(B_SH):
            a_n = acts.tile([128, 2, PADW], FP8, name=f"a{n}", tag=f"a{n}")
            nc.vector.memset(a_n[:, :, 0:30], 0.0)
            nc.vector.memset(a_n[:, :, 870:PADW], 0.0)
            pairs = a_n[:, :, 29 : 29 + 29 * HP].rearrange(
                "p cc (r two) -> p cc r two", two=HP
            )[:, :, :, :2]
            nc.vector.memset(pairs, 0.0)
            act_tiles.append(a_n)

        # ---- wsum = M + sum_k rv[k]*Z[k]; muls on ACT, add chain on DVE
        # (sequential k order to match the reference dot's rounding) ----
        # FMA chain, binarize, and transpose, pipelined by column halves
        # (half h of z_k arrives before the halves of z_{k+1})
        acc = wpool.tile([128, CT], F32, name="acc")
        w8 = wpool.tile([128, CT], FP8, name="w8")
        wt = consts.tile([128, KK, 2, 128], FP8, name="wt")
        halves = (slice(0, HCT), slice(HCT, CT))
        for h in range(2):
            sl = halves[h]
            nc.vector.tensor_scalar_mul(acc[:, sl], z_sb[0][:, sl], rv_sb[:, 0:1])
            for k in range(1, K):
                nc.vector.scalar_tensor_tensor(
                    acc[:, sl],
                    z_sb[k][:, sl],
                    rv_sb[:, k : k + 1],
                    acc[:, sl],
                    op0=mybir.AluOpType.mult,
                    op1=mybir.AluOpType.add,
                )
            nc.vector.tensor_add(acc[:, sl], acc[:, sl], m_sb[:, sl])
            nc.scalar.sign(w8[:, sl], acc[:, sl])
            cc = h  # c-chunk cc reads w8 columns [cc*1152, cc*1152+1152)
            for t in range(KK):
                blk = bass.AP(
                    tensor=w8.tensor,
                    offset=w8.offset + cc * 128 * KK + t,
                    ap=[w8.ap[0], [KK, 128]],
                )
                ps_t = pst.tile([128, 128], F32, name="ps_t", tag="ps_t")
                nc.tensor.matmul(ps_t, blk, eye8, start=True, stop=True)
                nc.vector.tensor_copy(wt[:, t, cc, :], ps_t)

        # ---- activations: sign(x) into the padded fp8 images (ACT) ----
        for n in range(B_SH):
            a_n = act_tiles[n]
            interior = a_n[:, :, 31 : 31 + 28 * HP].rearrange(
                "p cc (r xx) -> p cc r xx", xx=HP
            )[:, :, :, :28]
            nc.scalar.sign(interior, xst[n].rearrange("p cc (h w) -> p cc h w", w=28))

        # ---- conv: 9 taps x 2 half-images per image; both halves share
        # each tap's LDWEIGHTS (pair the matmuls) so weight loads hide ----
        for n in range(B_SH):
            a_n = act_tiles[n]
            ps0 = psums.tile([128, 420], F32, name="ps0", tag="ps")
            ps1 = psums.tile([128, 420], F32, name="ps1", tag="ps")
            pss = (ps0, ps1)
            for t in range(KK):
                dy, dx = divmod(t, 3)
                for half in range(2):
                    off = (half * 14 + dy) * HP + dx
                    nc.tensor.matmul(
                        pss[half],
                        wt[:, t],
                        a_n[:, :, off : off + 420],
                        start=(t == 0),
                        stop=(t == KK - 1),
                        perf_mode=mybir.MatmulPerfMode.DoubleRow,
                    )
            for half in range(2):
                ob = outp.tile([128, 392], F32, name="ob", tag="ob")
                ps_v = pss[half].rearrange("p (r xx) -> p r xx", xx=HP)[:, :, :28]
                ob_v = ob.rearrange("p (r xx) -> p r xx", xx=28)
                # alternate the psum-drain engine to balance ACT/DVE
                if half == 0:
                    nc.scalar.activation(
                        ob_v,
                        ps_v,
                        mybir.ActivationFunctionType.Copy,
                        scale=alpha_sb[:, 0:1],
                    )
                else:
                    nc.vector.tensor_scalar_mul(ob_v, ps_v, alpha_sb[:, 0:1])
                dst = out_t.ap()[n].rearrange("o h w -> o (h w)")[
                    :, half * 392 : (half + 1) * 392
                ]
                # out-writes ride the ACT HWDGE ring; the SP ring stays
                # dedicated to the input stream
                nc.scalar.dma_start(dst, ob)


_PROGRAM = None


def build_program():
    global _PROGRAM
    if _PROGRAM is not None:
        return _PROGRAM
    nc = bacc.Bacc(
        "TRN2",
        target_bir_lowering=False,
        debug=False,
        enable_asserts=True,
        num_devices=N_CORES,
    )
    x_t = nc.dram_tensor("x", [B_SH, C, H, H], F32, kind="ExternalInput")
    m_t = nc.dram_tensor("M", [O_SH, C, 3, 3], F32, kind="ExternalInput")
    z_t = nc.dram_tensor("Z", [K, O_SH, C, 3, 3], F32, kind="ExternalInput")
    a_t = nc.dram_tensor("alpha", [O_SH, 1, 1], F32, kind="ExternalInput")
    rv_t = nc.dram_tensor("rv", [1, K], F32, kind="ExternalInput")
    eye_t = nc.inline_tensor(np.eye(128, dtype=np.float32), name="eye128")
    ones_t = nc.inline_tensor(np.ones((1, 128), dtype=np.float32), name="ones128")
    out_t = nc.dram_tensor("out", [B_SH, O_SH, H, H], F32, kind="ExternalOutput")

    with tile.TileContext(nc) as tc:
        _build_kernel(tc, x_t, m_t, z_t, a_t, rv_t, eye_t, ones_t, out_t)
    nc.compile()
    _PROGRAM = nc
    return nc


def make_in_maps(x, M, Z, alpha, rv):
    x = np.ascontiguousarray(np.asarray(x, dtype=np.float32))
    M = np.ascontiguousarray(np.asarray(M, dtype=np.float32))
    Z = np.ascontiguousarray(np.asarray(Z, dtype=np.float32))
    alpha = np.ascontiguousarray(np.asarray(alpha, dtype=np.float32))
    rv = np.ascontiguousarray(np.asarray(rv, dtype=np.float32))
    in_maps = []
    for i in range(N_CORES):
        b, oh = divmod(i, 2)
        in_maps.append(
            {
                "x": np.ascontiguousarray(x[b * B_SH : (b + 1) * B_SH]),
                "M": np.ascontiguousarray(M[oh * O_SH : (oh + 1) * O_SH]),
                "Z": np.ascontiguousarray(Z[:, oh * O_SH : (oh + 1) * O_SH]),
                "alpha": np.ascontiguousarray(alpha[oh * O_SH : (oh + 1) * O_SH]),
                "rv": rv,
            }
        )
    return in_maps


def assemble_out(results):
    out = np.empty((B, O, H, H), dtype=np.float32)
    for i in range(N_CORES):
        b, oh = divmod(i, 2)
        r = np.asarray(results[i]["out"]).reshape(B_SH, O_SH, H, H)
        out[b * B_SH : (b + 1) * B_SH, oh * O_SH : (oh + 1) * O_SH] = r
    return out


def kernel(x, M, Z, alpha, rv, trace=False):
    nc = build_program()
    in_maps = make_in_maps(x, M, Z, alpha, rv)
    res = run_bass_kernel_spmd(
        nc, in_maps, core_ids=list(range(N_CORES)), trace=trace
    )
    if trace:
        kernel.last_results = res
    return assemble_out(res.results)


if __name__ == "__main__":
    build_program()
    print("program built ok")


# revision 26
# speedup vs baseline: 1.1079x; 1.0014x over previous
"""Trainium2 Bass kernel for BinarizeConv2dSDP.

Reference math (forward only):
    w    = rsqrt(m^2 + sum_k z_k^2/100) * (m + rv @ z)   elementwise
    bw   = sign(w)        -- the positive rsqrt factor drops out of sign()
    ba   = sign(x)
    out  = conv2d(ba, bw, pad=1, NCHW/OIHW) * alpha[o]

Device computation: bw = sign(M + sum_k rv[k]*Z[k]), ba = sign(x), then the
3x3 pad-1 conv as 9 shifted fp8 DoubleRow matmuls accumulating in PSUM
(everything is +-1, so fp8 e4m3 with f32 PSUM accumulation is bit-exact),
alpha folded into the PSUM->SBUF copy.

Sharding (8 cores, no collectives): 2D grid, batch 4-way x out-channel
2-way. Core i handles images [16*(i//2), 16*(i//2)+16) and out-channels
[128*(i%2), 128*(i%2)+128). Each core reads only its Z/M/alpha o-half and
its x batch-quarter; outputs are disjoint.

Per-core layouts:
  - z_k, m, wsum: [128 part(o), 2304 (c*9+t)] f32 -- natural Z order, so
    all weight DMAs are fully contiguous.
  - weight sum on ACT (muls by rv[k]) + DVE (sequential add chain, same
    f32 order as the reference dot).
  - sign -> w8 [128(o), 2304] fp8; 18 PE transposes (matmul with fp8
    identity rhs, lhsT = stride-9 column slice) -> W [128 part(c_low),
    9 tap, 2 c-chunk, 128 o] fp8.
  - activations: per image [128 part(c_low), 2 c-chunk, 912] fp8 zero-
    padded 30x30 images (912 = 900 rounded up to %16 for the DoubleRow
    AP-step constraint); conv output on a 30-wide grid, junk columns
    skipped at the output DMA.
"""

import sys

for _p in ("/opt/trn_rl_repo",):
    if _p not in sys.path:
        sys.path.insert(0, _p)

import contextlib

import numpy as np

import concourse.bass as bass
import concourse.bacc as bacc
import concourse.tile as tile
from concourse import mybir
from concourse.bass_utils import run_bass_kernel_spmd

N_CORES = 8
B = 64
B_SH = 16       # images per core (batch/4)
C = 256         # in channels
O = 256
O_SH = 128      # out channels per core (o/2)
K = 8           # SDP rank
KK = 9          # 3x3 taps
CT = C * KK     # 2304
H = 28
HP = 30         # padded row width
PADW = 912      # 30*30=900 padded to %16
F32 = mybir.dt.float32
FP8 = mybir.dt.float8e4


def _build_kernel(tc, x_t, m_t, z_t, a_t, rv_t, eye_t, ones_t, out_t):
    nc = tc.nc
    ctx = contextlib.ExitStack()
    consts = ctx.enter_context(tc.tile_pool(name="consts", bufs=1))
    zpool = ctx.enter_context(tc.tile_pool(name="zpool", bufs=1))
    wpool = ctx.enter_context(tc.tile_pool(name="wpool", bufs=1))
    stage = ctx.enter_context(tc.tile_pool(name="stage", bufs=8))
    acts = ctx.enter_context(tc.tile_pool(name="acts", bufs=1))
    outp = ctx.enter_context(tc.tile_pool(name="outp", bufs=8))
    psums = ctx.enter_context(tc.tile_pool(name="psums", bufs=6, space="PSUM"))
    pst = ctx.enter_context(tc.tile_pool(name="pst", bufs=2, space="PSUM"))

    with ctx:
        # ---- tiny constants. rv is partition-broadcast via a K=1 matmul
        # (ones.T @ rv) on the otherwise-idle PE — a [0,128]-step broadcast
        # DMA would stall its queue with 128 tiny descriptors. ----
        rv_raw = consts.tile([1, K], F32, name="rv_raw")
        nc.sync.dma_start(rv_raw, rv_t.ap())
        ones_sb = consts.tile([1, 128], F32, name="ones_sb")
        nc.sync.dma_start(ones_sb, ones_t.ap())
        alpha_sb = consts.tile([128, 1], F32, name="alpha_sb")
        nc.gpsimd.dma_start(alpha_sb, a_t.ap().rearrange("p a b -> p (a b)"))
        ps_rv = pst.tile([128, K], F32, name="ps_rv", tag="ps_t")
        nc.tensor.matmul(ps_rv, ones_sb, rv_raw, start=True, stop=True)
        rv_sb = consts.tile([128, K], F32, name="rv_sb")
        nc.vector.tensor_copy(rv_sb, ps_rv)
        eye_sb = consts.tile([128, 128], F32, name="eye_sb")
        nc.gpsimd.dma_start(eye_sb, eye_t.ap())
        eye8 = consts.tile([128, 128], FP8, name="eye8")
        nc.scalar.sign(eye8, eye_sb)

        # ---- weight inputs first: fully contiguous [o, c*9+t] loads.
        # Everything big goes on the single SP HWDGE ring, in priority
        # order (z+M gate the conv start; x streams behind them). ----
        HCT = CT // 2
        m_sb = zpool.tile([128, CT], F32, name="m_sb")
        nc.sync.dma_start(m_sb, m_t.ap().rearrange("o c ky kx -> o (c ky kx)"))
        z_sb = []
        for k in range(K):
            z_k = zpool.tile([128, CT], F32, name=f"z{k}", tag="z", bufs=5)
            z_src = z_t.ap()[k].rearrange("o c ky kx -> o (c ky kx)")
            for h in range(2):
                sl = slice(h * HCT, (h + 1) * HCT)
                nc.sync.dma_start(z_k[:, sl], z_src[:, sl])
            z_sb.append(z_k)

        xst = []
        for n in range(B_SH):
            xst.append(stage.tile([128, 2, H * H], F32, name=f"xst{n}", tag="xst"))
        for n in range(B_SH):
            nc.sync.dma_start(
                xst[n], x_t.ap()[n].rearrange("(cc p) h w -> p cc (h w)", p=128)
            )

        # ---- wsum = M + sum_k rv[k]*Z[k]: fused-FMA chain on DVE
        # (sequential k order, same f32 rounding as the reference dot),
        # binarize + transpose pipelined by column halves ----
        acc = wpool.tile([128, CT], F32, name="acc")
        w8 = wpool.tile([128, CT], FP8, name="w8")
        wt = consts.tile([128, KK, 2, 128], FP8, name="wt")
        halves = (slice(0, HCT), slice(HCT, CT))
        for h in range(2):
            sl = halves[h]
            nc.vector.tensor_scalar_mul(acc[:, sl], z_sb[0][:, sl], rv_sb[:, 0:1])
            for k in range(1, K):
                nc.vector.scalar_tensor_tensor(
                    acc[:, sl],
                    z_sb[k][:, sl],
                    rv_sb[:, k : k + 1],
                    acc[:, sl],
                    op0=mybir.AluOpType.mult,
                    op1=mybir.AluOpType.add,
                )
            nc.vector.tensor_add(acc[:, sl], acc[:, sl], m_sb[:, sl])
            nc.scalar.sign(w8[:, sl], acc[:, sl])
            cc = h  # c-chunk cc reads w8 columns [cc*1152, cc*1152+1152)
            for t in range(KK):
                blk = bass.AP(
                    tensor=w8.tensor,
                    offset=w8.offset + cc * 128 * KK + t,
                    ap=[w8.ap[0], [KK, 128]],
                )
                ps_t = pst.tile([128, 128], F32, name="ps_t", tag="ps_t")
                nc.tensor.matmul(ps_t, blk, eye8, start=True, stop=True)
                nc.vector.tensor_copy(wt[:, t, cc, :], ps_t)

        # ---- activations: zero the padding borders (DVE; emitted after
        # the weight chain so they don't delay it), sign(x) on ACT ----
        act_tiles = []
        for n in range(B_SH):
            a_n = acts.tile([128, 2, PADW], FP8, name=f"a{n}", tag=f"a{n}")
            nc.vector.memset(a_n[:, :, 0:30], 0.0)
            nc.vector.memset(a_n[:, :, 870:PADW], 0.0)
            pairs = a_n[:, :, 29 : 29 + 29 * HP].rearrange(
                "p cc (r two) -> p cc r two", two=HP
            )[:, :, :, :2]
            nc.vector.memset(pairs, 0.0)
            interior = a_n[:, :, 31 : 31 + 28 * HP].rearrange(
                "p cc (r xx) -> p cc r xx", xx=HP
            )[:, :, :, :28]
            nc.scalar.sign(interior, xst[n].rearrange("p cc (h w) -> p cc h w", w=28))
            act_tiles.append(a_n)

        # ---- conv: 9 taps x 2 half-images per image; both halves share
        # each tap's LDWEIGHTS (pair the matmuls) so weight loads hide ----
        for n in range(B_SH):
            a_n = act_tiles[n]
            ps0 = psums.tile([128, 420], F32, name="ps0", tag="ps")
            ps1 = psums.tile([128, 420], F32, name="ps1", tag="ps")
            pss = (ps0, ps1)
            for t in range(KK):
                dy, dx = divmod(t, 3)
                for half in range(2):
                    off = (half * 14 + dy) * HP + dx
                    nc.tensor.matmul(
                        pss[half],
                        wt[:, t],
                        a_n[:, :, off : off + 420],
                        start=(t == 0),
                        stop=(t == KK - 1),
                        perf_mode=mybir.MatmulPerfMode.DoubleRow,
                    )
            for half in range(2):
                ob = outp.tile([128, 392], F32, name="ob", tag="ob")
                ps_v = pss[half].rearrange("p (r xx) -> p r xx", xx=HP)[:, :, :28]
                ob_v = ob.rearrange("p (r xx) -> p r xx", xx=28)
                # alternate the psum-drain engine to balance ACT/DVE
                if half == 0:
                    nc.scalar.activation(
                        ob_v,
                        ps_v,
                        mybir.ActivationFunctionType.Copy,
                        scale=alpha_sb[:, 0:1],
                    )
                else:
                    nc.vector.tensor_scalar_mul(ob_v, ps_v, alpha_sb[:, 0:1])
                dst = out_t.ap()[n].rearrange("o h w -> o (h w)")[
                    :, half * 392 : (half + 1) * 392
                ]
                # out-writes ride the ACT HWDGE ring; the SP ring stays
                # dedicated to the input stream
                nc.scalar.dma_start(dst, ob)


_PROGRAM = None


def build_program():
    global _PROGRAM
    if _PROGRAM is not None:
        return _PROGRAM
    nc = bacc.Bacc(
        "TRN2",
        target_bir_lowering=False,
        debug=False,
        enable_asserts=True,
        num_devices=N_CORES,
    )
    x_t = nc.dram_tensor("x", [B_SH, C, H, H], F32, kind="ExternalInput")
    m_t = nc.dram_tensor("M", [O_SH, C, 3, 3], F32, kind="ExternalInput")
    z_t = nc.dram_tensor("Z", [K, O_SH, C, 3, 3], F32, kind="ExternalInput")
    a_t = nc.dram_tensor("alpha", [O_SH, 1, 1], F32, kind="ExternalInput")
    rv_t = nc.dram_tensor("rv", [1, K], F32, kind="ExternalInput")
    eye_t = nc.inline_tensor(np.eye(128, dtype=np.float32), name="eye128")
    ones_t = nc.inline_tensor(np.ones((1, 128), dtype=np.float32), name="ones128")
    out_t = nc.dram_tensor("out", [B_SH, O_SH, H, H], F32, kind="ExternalOutput")

    with tile.TileContext(nc) as tc:
        _build_kernel(tc, x_t, m_t, z_t, a_t, rv_t, eye_t, ones_t, out_t)
    nc.compile()
    _PROGRAM = nc
    return nc


def make_in_maps(x, M, Z, alpha, rv):
    x = np.ascontiguousarray(np.asarray(x, dtype=np.float32))
    M = np.ascontiguousarray(np.asarray(M, dtype=np.float32))
    Z = np.ascontiguousarray(np.asarray(Z, dtype=np.float32))
    alpha = np.ascontiguousarray(np.asarray(alpha, dtype=np.float32))
    rv = np.ascontiguousarray(np.asarray(rv, dtype=np.float32))
    in_maps = []
    for i in range(N_CORES):
        b, oh = divmod(i, 2)
        in_maps.append(
            {
                "x": np.ascontiguousarray(x[b * B_SH : (b + 1) * B_SH]),
                "M": np.ascontiguousarray(M[oh * O_SH : (oh + 1) * O_SH]),
                "Z": np.ascontiguousarray(Z[:, oh * O_SH : (oh + 1) * O_SH]),
                "alpha": np.ascontiguousarray(alpha[oh * O_SH : (oh + 1) * O_SH]),
                "rv": rv,
            }
        )
    return in_maps


def assemble_out(results):
    out = np.empty((B, O, H, H), dtype=np.float32)
    for i in range(N_CORES):
        b, oh = divmod(i, 2)
        r = np.asarray(results[i]["out"]).reshape(B_SH, O_SH, H, H)
        out[b * B_SH : (b + 1) * B_SH, oh * O_SH : (oh + 1) * O_SH] = r
    return out


def kernel(x, M, Z, alpha, rv, trace=False):
    nc = build_program()
    in_maps = make_in_maps(x, M, Z, alpha, rv)
    res = run_bass_kernel_spmd(
        nc, in_maps, core_ids=list(range(N_CORES)), trace=trace
    )
    if trace:
        kernel.last_results = res
    return assemble_out(res.results)


if __name__ == "__main__":
    build_program()
    print("program built ok")


# revision 29
# speedup vs baseline: 1.1117x; 1.0034x over previous
"""Trainium2 Bass kernel for BinarizeConv2dSDP.

Reference math (forward only):
    w    = rsqrt(m^2 + sum_k z_k^2/100) * (m + rv @ z)   elementwise
    bw   = sign(w)        -- the positive rsqrt factor drops out of sign()
    ba   = sign(x)
    out  = conv2d(ba, bw, pad=1, NCHW/OIHW) * alpha[o]

Device computation: bw = sign(M + sum_k rv[k]*Z[k]), ba = sign(x), then the
3x3 pad-1 conv as 9 shifted fp8 DoubleRow matmuls accumulating in PSUM
(everything is +-1, so fp8 e4m3 with f32 PSUM accumulation is bit-exact),
alpha folded into the PSUM->SBUF copy.

Sharding (8 cores, no collectives): 2D grid, batch 4-way x out-channel
2-way. Core i handles images [16*(i//2), 16*(i//2)+16) and out-channels
[128*(i%2), 128*(i%2)+128). Each core reads only its Z/M/alpha o-half and
its x batch-quarter; outputs are disjoint.

Per-core layouts:
  - z_k, m, wsum: [128 part(o), 2304 (c*9+t)] f32 -- natural Z order, so
    all weight DMAs are fully contiguous.
  - weight sum on ACT (muls by rv[k]) + DVE (sequential add chain, same
    f32 order as the reference dot).
  - sign -> w8 [128(o), 2304] fp8; 18 PE transposes (matmul with fp8
    identity rhs, lhsT = stride-9 column slice) -> W [128 part(c_low),
    9 tap, 2 c-chunk, 128 o] fp8.
  - activations: per image [128 part(c_low), 2 c-chunk, 912] fp8 zero-
    padded 30x30 images (912 = 900 rounded up to %16 for the DoubleRow
    AP-step constraint); conv output on a 30-wide grid, junk columns
    skipped at the output DMA.
"""

import sys

for _p in ("/opt/trn_rl_repo",):
    if _p not in sys.path:
        sys.path.insert(0, _p)

import contextlib

import numpy as np

import concourse.bass as bass
import concourse.bacc as bacc
import concourse.tile as tile
from concourse import mybir
from concourse.bass_utils import run_bass_kernel_spmd

N_CORES = 8
B = 64
B_SH = 16       # images per core (batch/4)
C = 256         # in channels
O = 256
O_SH = 128      # out channels per core (o/2)
K = 8           # SDP rank
KK = 9          # 3x3 taps
CT = C * KK     # 2304
H = 28
HP = 30         # padded row width
PADW = 912      # 30*30=900 padded to %16
F32 = mybir.dt.float32
FP8 = mybir.dt.float8e4


def _build_kernel(tc, x_t, m_t, z_t, a_t, rv_t, eye_t, ones_t, out_t):
    nc = tc.nc
    ctx = contextlib.ExitStack()
    consts = ctx.enter_context(tc.tile_pool(name="consts", bufs=1))
    zpool = ctx.enter_context(tc.tile_pool(name="zpool", bufs=1))
    wpool = ctx.enter_context(tc.tile_pool(name="wpool", bufs=1))
    stage = ctx.enter_context(tc.tile_pool(name="stage", bufs=8))
    acts = ctx.enter_context(tc.tile_pool(name="acts", bufs=1))
    outp = ctx.enter_context(tc.tile_pool(name="outp", bufs=8))
    psums = ctx.enter_context(tc.tile_pool(name="psums", bufs=6, space="PSUM"))
    pst = ctx.enter_context(tc.tile_pool(name="pst", bufs=2, space="PSUM"))

    with ctx:
        # ---- tiny constants. rv is partition-broadcast via a K=1 matmul
        # (ones.T @ rv) on the otherwise-idle PE — a [0,128]-step broadcast
        # DMA would stall its queue with 128 tiny descriptors. ----
        rv_raw = consts.tile([1, K], F32, name="rv_raw")
        nc.sync.dma_start(rv_raw, rv_t.ap())
        ones_sb = consts.tile([1, 128], F32, name="ones_sb")
        nc.sync.dma_start(ones_sb, ones_t.ap())
        alpha_sb = consts.tile([128, 1], F32, name="alpha_sb")
        nc.gpsimd.dma_start(alpha_sb, a_t.ap().rearrange("p a b -> p (a b)"))
        ps_rv = pst.tile([128, K], F32, name="ps_rv", tag="ps_t")
        nc.tensor.matmul(ps_rv, ones_sb, rv_raw, start=True, stop=True)
        rv_sb = consts.tile([128, K], F32, name="rv_sb")
        nc.vector.tensor_copy(rv_sb, ps_rv)
        eye_sb = consts.tile([128, 128], F32, name="eye_sb")
        nc.gpsimd.dma_start(eye_sb, eye_t.ap())
        eye8 = consts.tile([128, 128], FP8, name="eye8")
        nc.scalar.sign(eye8, eye_sb)

        # ---- weight inputs first: fully contiguous [o, c*9+t] loads.
        # Everything big goes on the single SP HWDGE ring, in priority
        # order (z+M gate the conv start; x streams behind them). ----
        HCT = CT // 2
        m_sb = zpool.tile([128, CT], F32, name="m_sb")
        nc.sync.dma_start(m_sb, m_t.ap().rearrange("o c ky kx -> o (c ky kx)"))
        z_sb = []
        for k in range(K):
            z_k = zpool.tile([128, CT], F32, name=f"z{k}", tag="z", bufs=5)
            z_src = z_t.ap()[k].rearrange("o c ky kx -> o (c ky kx)")
            for h in range(2):
                sl = slice(h * HCT, (h + 1) * HCT)
                nc.sync.dma_start(z_k[:, sl], z_src[:, sl])
            z_sb.append(z_k)

        xst = []
        for n in range(B_SH):
            xst.append(stage.tile([128, 2, H * H], F32, name=f"xst{n}", tag="xst"))
        for n in range(B_SH):
            nc.sync.dma_start(
                xst[n], x_t.ap()[n].rearrange("(cc p) h w -> p cc (h w)", p=128)
            )

        # ---- wsum = M + sum_k rv[k]*Z[k]: fused-FMA chain on DVE
        # (sequential k order, same f32 rounding as the reference dot),
        # binarize + transpose pipelined by column halves ----
        acc = wpool.tile([128, CT], F32, name="acc")
        w8 = wpool.tile([128, CT], FP8, name="w8")
        wt = consts.tile([128, KK, 2, 128], FP8, name="wt")
        halves = (slice(0, HCT), slice(HCT, CT))
        for h in range(2):
            sl = halves[h]
            # acc = z0*rv0 + m, then acc += z_k*rv_k -- one fused op per k
            nc.vector.scalar_tensor_tensor(
                acc[:, sl],
                z_sb[0][:, sl],
                rv_sb[:, 0:1],
                m_sb[:, sl],
                op0=mybir.AluOpType.mult,
                op1=mybir.AluOpType.add,
            )
            for k in range(1, K):
                nc.vector.scalar_tensor_tensor(
                    acc[:, sl],
                    z_sb[k][:, sl],
                    rv_sb[:, k : k + 1],
                    acc[:, sl],
                    op0=mybir.AluOpType.mult,
                    op1=mybir.AluOpType.add,
                )
            nc.scalar.sign(w8[:, sl], acc[:, sl])
            cc = h  # c-chunk cc reads w8 columns [cc*1152, cc*1152+1152)
            for t in range(KK):
                blk = bass.AP(
                    tensor=w8.tensor,
                    offset=w8.offset + cc * 128 * KK + t,
                    ap=[w8.ap[0], [KK, 128]],
                )
                ps_t = pst.tile([128, 128], F32, name="ps_t", tag="ps_t")
                nc.tensor.matmul(ps_t, blk, eye8, start=True, stop=True)
                nc.vector.tensor_copy(wt[:, t, cc, :], ps_t)

        # ---- activations: zero the padding borders (DVE; emitted after
        # the weight chain so they don't delay it), sign(x) on ACT ----
        act_tiles = []
        for n in range(B_SH):
            a_n = acts.tile([128, 2, PADW], FP8, name=f"a{n}", tag=f"a{n}")
            nc.vector.memset(a_n[:, :, 0:30], 0.0)
            nc.vector.memset(a_n[:, :, 870:PADW], 0.0)
            pairs = a_n[:, :, 29 : 29 + 29 * HP].rearrange(
                "p cc (r two) -> p cc r two", two=HP
            )[:, :, :, :2]
            nc.vector.memset(pairs, 0.0)
            interior = a_n[:, :, 31 : 31 + 28 * HP].rearrange(
                "p cc (r xx) -> p cc r xx", xx=HP
            )[:, :, :, :28]
            nc.scalar.sign(interior, xst[n].rearrange("p cc (h w) -> p cc h w", w=28))
            act_tiles.append(a_n)

        # ---- conv: 9 taps x 2 half-images per image; both halves share
        # each tap's LDWEIGHTS (pair the matmuls) so weight loads hide ----
        for n in range(B_SH):
            a_n = act_tiles[n]
            ps0 = psums.tile([128, 420], F32, name="ps0", tag="ps")
            ps1 = psums.tile([128, 420], F32, name="ps1", tag="ps")
            pss = (ps0, ps1)
            for t in range(KK):
                dy, dx = divmod(t, 3)
                for half in range(2):
                    off = (half * 14 + dy) * HP + dx
                    nc.tensor.matmul(
                        pss[half],
                        wt[:, t],
                        a_n[:, :, off : off + 420],
                        start=(t == 0),
                        stop=(t == KK - 1),
                        perf_mode=mybir.MatmulPerfMode.DoubleRow,
                    )
            for half in range(2):
                ob = outp.tile([128, 392], F32, name="ob", tag="ob")
                ps_v = pss[half].rearrange("p (r xx) -> p r xx", xx=HP)[:, :, :28]
                ob_v = ob.rearrange("p (r xx) -> p r xx", xx=28)
                # alternate the psum-drain engine to balance ACT/DVE
                if half == 0:
                    nc.scalar.activation(
                        ob_v,
                        ps_v,
                        mybir.ActivationFunctionType.Copy,
                        scale=alpha_sb[:, 0:1],
                    )
                else:
                    nc.vector.tensor_scalar_mul(ob_v, ps_v, alpha_sb[:, 0:1])
                dst = out_t.ap()[n].rearrange("o h w -> o (h w)")[
                    :, half * 392 : (half + 1) * 392
                ]
                # out-writes ride the ACT HWDGE ring; the SP ring stays
                # dedicated to the input stream
                nc.scalar.dma_start(dst, ob)


_PROGRAM = None


def build_program():
    global _PROGRAM
    if _PROGRAM is not None:
        return _PROGRAM
    nc = bacc.Bacc(
        "TRN2",
        target_bir_lowering=False,
        debug=False,
        enable_asserts=True,
        num_devices=N_CORES,
    )
    x_t = nc.dram_tensor("x", [B_SH, C, H, H], F32, kind="ExternalInput")
    m_t = nc.dram_tensor("M", [O_SH, C, 3, 3], F32, kind="ExternalInput")
    z_t = nc.dram_tensor("Z", [K, O_SH, C, 3, 3], F32, kind="ExternalInput")
    a_t = nc.dram_tensor("alpha", [O_SH, 1, 1], F32, kind="ExternalInput")
    rv_t = nc.dram_tensor("rv", [1, K], F32, kind="ExternalInput")
    eye_t = nc.inline_tensor(np.eye(128, dtype=np.float32), name="eye128")
    ones_t = nc.inline_tensor(np.ones((1, 128), dtype=np.float32), name="ones128")
    out_t = nc.dram_tensor("out", [B_SH, O_SH, H, H], F32, kind="ExternalOutput")

    with tile.TileContext(nc) as tc:
        _build_kernel(tc, x_t, m_t, z_t, a_t, rv_t, eye_t, ones_t, out_t)
    nc.compile()
    _PROGRAM = nc
    return nc


def make_in_maps(x, M, Z, alpha, rv):
    x = np.ascontiguousarray(np.asarray(x, dtype=np.float32))
    M = np.ascontiguousarray(np.asarray(M, dtype=np.float32))
    Z = np.ascontiguousarray(np.asarray(Z, dtype=np.float32))
    alpha = np.ascontiguousarray(np.asarray(alpha, dtype=np.float32))
    rv = np.ascontiguousarray(np.asarray(rv, dtype=np.float32))
    in_maps = []
    for i in range(N_CORES):
        b, oh = divmod(i, 2)
        in_maps.append(
            {
                "x": np.ascontiguousarray(x[b * B_SH : (b + 1) * B_SH]),
                "M": np.ascontiguousarray(M[oh * O_SH : (oh + 1) * O_SH]),
                "Z": np.ascontiguousarray(Z[:, oh * O_SH : (oh + 1) * O_SH]),
                "alpha": np.ascontiguousarray(alpha[oh * O_SH : (oh + 1) * O_SH]),
                "rv": rv,
            }
        )
    return in_maps


def assemble_out(results):
    out = np.empty((B, O, H, H), dtype=np.float32)
    for i in range(N_CORES):
        b, oh = divmod(i, 2)
        r = np.asarray(results[i]["out"]).reshape(B_SH, O_SH, H, H)
        out[b * B_SH : (b + 1) * B_SH, oh * O_SH : (oh + 1) * O_SH] = r
    return out


def kernel(x, M, Z, alpha, rv, trace=False):
    nc = build_program()
    in_maps = make_in_maps(x, M, Z, alpha, rv)
    res = run_bass_kernel_spmd(
        nc, in_maps, core_ids=list(range(N_CORES)), trace=trace
    )
    if trace:
        kernel.last_results = res
    return assemble_out(res.results)


if __name__ == "__main__":
    build_program()
    print("program built ok")
